# revision 14
# baseline (speedup 1.0000x reference)
"""Trainium2 Bass kernel for nn_Attention (sparse_attention, T=3).

Math (per batch row b, derived from the reference):
    zq = z[:, :3*2048].reshape(B, 3, D)   (q and v source)
    zk = z[:, 3*2048:].reshape(B, 3, D)
    scores[t,s] = (zq[t] @ M @ zk[s] + zq[t].a + r.zk[s] + kap) / sqrt(D)
      with M = wq.T @ wk, a = wq.T @ bk, r = bq @ wk, kap = bq.bk
    strictly-lower entries of scores are replaced by 0 before softmax
    p = softmax(scores); w[s] = sum_t p[t,s]
    y = (sum_s w[s]*zq[s]) @ Q + c,  Q = wv.T @ wo.T,  c = 3*bv@wo.T + 3*bo

Distribution: data-parallel over batch (B/8 rows per core), plus the two
DxD weight products M and Q are column-sharded: core c computes the
256-wide e-slice it was fed (host passes wk[:, c*256:+256] and
wo[c*256:+256, :]), then an 8-core AllGather replicates the full M and
Q in bf16. Score dots are fused multiply-reduce (DVE) reading the G
psums directly; the value path is a single matmul against Q.
"""

import sys

sys.path.insert(0, "/opt/trn_rl_repo")

import numpy as np
from concourse import bacc, bass, masks, mybir, tile
from concourse.bass_utils import run_bass_kernel_spmd

F32 = mybir.dt.float32
BF16 = mybir.dt.bfloat16
FP8 = mybir.dt.float8e4
ADD = mybir.AluOpType.add
MULT = mybir.AluOpType.mult
CPY = mybir.ActivationFunctionType.Copy
EXP = mybir.ActivationFunctionType.Exp

B = 8192
D = 2048
T = 3
NCORES = 8
DC = D // 128       # 16 chunks of the feature dim
EC = D // 512       # 4 psum-wide slices
ES = D // NCORES    # 256: per-core e-shard width
HD = D // 2
SQD = 1.0 / float(np.sqrt(np.float32(D)))
MSC = 32.0          # fp8 scaling for the a vector


def emit(tc, aps, b_loc):
    nc = tc.nc
    z, wq, wk_sl, wv, wo_sl = (aps["z"], aps["wq"], aps["wk_sl"],
                               aps["wv"], aps["wo_sl"])
    bq, bk, bv, bo_sl, out = (aps["bq"], aps["bk"], aps["bv"], aps["bo_sl"],
                              aps["out"])
    NB = b_loc // 128

    const = tc.alloc_tile_pool(name="const", bufs=1)
    persist = tc.alloc_tile_pool(name="persist", bufs=1)

    ident = const.tile([128, 128], BF16)
    masks.make_identity(nc, ident[:])

    # biases in column layout: col[p, c] = vec[c*128 + p]
    bq_col = const.tile([128, DC], F32)
    bk_col = const.tile([128, DC], F32)
    bv_col = const.tile([128, DC], F32)
    bo_sl_row = const.tile([1, ES], F32)
    nc.sync.dma_start(bq_col[:], bq.rearrange("(c p) -> p c", p=128))
    nc.sync.dma_start(bk_col[:], bk.rearrange("(c p) -> p c", p=128))
    nc.sync.dma_start(bv_col[:], bv.rearrange("(c p) -> p c", p=128))
    nc.sync.dma_start(bo_sl_row[:], bo_sl[None, :])
    bq_colbf = const.tile([128, DC], BF16)
    bk_colbf = const.tile([128, DC], BF16)
    bv_colbf = const.tile([128, DC], BF16)
    nc.vector.tensor_copy(bq_colbf[:], bq_col[:])
    nc.vector.tensor_copy(bk_colbf[:], bk_col[:])
    nc.vector.tensor_copy(bv_colbf[:], bv_col[:])

    # broadcast score-bias tensors (filled in phase 1)
    a_rep = persist.tile([128, D], FP8)     # 32*a = 32*wq.T@bk, replicated
    r_rep = persist.tile([128, D], BF16)    # r = bq@wk, replicated
    c_rep = persist.tile([128, D], BF16)    # c = 3bv@wo.T + 3bo, replicated
    kap_col = persist.tile([128, 1], F32)   # bq.bk / sqrt(D)

    m_bf = persist.tile([128, DC, D], BF16)  # M[d, e], partition = d%128
    qpool = tc.alloc_tile_pool(name="qpool", bufs=1, side="right")
    q_bf = qpool.tile([128, DC, D], BF16)   # Q[j, e], partition = j%128

    # ------------- Phase 1: sharded M/Q products + AllGather ---------------
    dram = tc.alloc_tile_pool(name="dram", bufs=1, space="DRAM")
    bnc1 = dram.tile([DC * 128 + 1, ES], BF16)
    ag1 = dram.tile([NCORES, DC * 128 + 1, ES], BF16, addr_space="Shared")
    bnc2 = dram.tile([DC * 128 + 1, ES], BF16)
    ag2 = dram.tile([NCORES, DC * 128 + 1, ES], BF16, addr_space="Shared")

    with (
        tc.tile_pool(name="p1_w", bufs=1) as p_w,
        tc.tile_pool(name="p1_io", bufs=2) as p_io,
        tc.tile_pool(name="p1_sm", bufs=1) as p_sm,
        tc.tile_pool(name="p1_ps", bufs=3, space="PSUM") as pp_m,
        tc.tile_pool(name="p1_psr", bufs=1, space="PSUM") as pp_r,
    ):
        # wk shard in bf16 (rhs)
        wk_bf = p_sm.tile([128, DC, ES], BF16, tag="wsl", name="wk_bf")
        for n in range(DC):
            wk_f = p_io.tile([128, ES], F32, tag="wkload", bufs=2)
            nc.sync.dma_start(wk_f[:], wk_sl[n * 128:(n + 1) * 128, :])
            nc.scalar.activation(wk_bf[:, n, :], wk_f[:], CPY)

        # M shard: M[d, e_sl] = sum_n wq[n, d] wk[n, e_sl].
        # wq is loaded in two column halves (lhsT free dim) to halve SBUF.
        a_row = p_sm.tile([1, D], FP8, name="a_row")
        for h in range(2):
            wq_bf = p_w.tile([128, DC, HD], BF16, tag="wbig", name="wq_bf")
            for n in range(DC):
                wq_f = p_io.tile([128, HD], F32, tag="wload", bufs=2)
                nc.sync.dma_start(
                    wq_f[:], wq[n * 128:(n + 1) * 128, h * HD:(h + 1) * HD])
                if n % 2 == 0:
                    nc.scalar.activation(wq_bf[:, n, :], wq_f[:], CPY)
                else:
                    nc.vector.tensor_copy(wq_bf[:, n, :], wq_f[:])
            for dh in range(DC // 2):
                dc = h * (DC // 2) + dh
                ps = pp_m.tile([128, ES], F32)
                for n in range(DC):
                    nc.tensor.matmul(ps[:],
                                     wq_bf[:, n, dh * 128:(dh + 1) * 128],
                                     wk_bf[:, n, :], start=(n == 0),
                                     stop=(n == DC - 1))
                mstg = p_sm.tile([128, ES], BF16, tag="stg", bufs=2,
                                 name="mstg")
                nc.vector.tensor_copy(mstg[:], ps[:])
                nc.sync.dma_start(bnc1[dc * 128:(dc + 1) * 128, :], mstg[:])
            # a = wq.T @ bk for this half (x32, fp8)
            for e in range(EC // 2):
                ps_a = pp_r.tile([1, 512], F32, tag="psa")
                for n in range(DC):
                    nc.tensor.matmul(ps_a[:], bk_colbf[:, n:n + 1],
                                     wq_bf[:, n, e * 512:(e + 1) * 512],
                                     start=(n == 0), stop=(n == DC - 1))
                nc.vector.tensor_scalar(
                    a_row[:, h * HD + e * 512:h * HD + (e + 1) * 512],
                    ps_a[:], MSC, None, op0=MULT)
        # r shard = bq @ wk
        ps_r = pp_r.tile([1, ES], F32, tag="ps256", name="ps_r")
        for n in range(DC):
            nc.tensor.matmul(ps_r[:], bq_colbf[:, n:n + 1], wk_bf[:, n, :],
                             start=(n == 0), stop=(n == DC - 1))
        rstg = p_sm.tile([1, ES], BF16)
        nc.vector.tensor_copy(rstg[:], ps_r[:])
        nc.sync.dma_start(bnc1[DC * 128:, :], rstg[:])
        nc.gpsimd.collective_compute(
            "AllGather", mybir.AluOpType.bypass,
            replica_groups=[list(range(NCORES))],
            ins=[bnc1[:]], outs=[ag1[:]])

        nc.gpsimd.partition_broadcast(a_rep[:], a_row[:])
        ps_k = pp_r.tile([1, ES], F32, tag="ps256", name="ps_k")
        for n in range(DC):
            nc.tensor.matmul(ps_k[:, 0:1], bq_colbf[:, n:n + 1],
                             bk_colbf[:, n:n + 1],
                             start=(n == 0), stop=(n == DC - 1))
        kap_row = p_sm.tile([1, 1], F32)
        nc.vector.tensor_copy(kap_row[:], ps_k[:, 0:1])
        nc.gpsimd.partition_broadcast(kap_col[:], kap_row[:])
        nc.vector.tensor_scalar(kap_col[:], kap_col[:], SQD, None, op0=MULT)

        # wo shard rows, transposed on PE: woT[i, e_sl]
        woT = p_sm.tile([128, DC, ES], BF16, tag="wsl", name="woT")
        for ec in range(ES // 128):
            wo_f = p_io.tile([128, D], F32, tag="wold", bufs=1)
            nc.sync.dma_start(wo_f[:], wo_sl[ec * 128:(ec + 1) * 128, :])
            wo_b = p_io.tile([128, D], BF16, tag="wob", bufs=1)
            nc.scalar.activation(wo_b[:], wo_f[:], CPY)
            for ic in range(DC):
                ps = pp_m.tile([128, 128], BF16, tag="pst", bufs=2)
                nc.tensor.matmul(ps[:], wo_b[:, ic * 128:(ic + 1) * 128],
                                 ident[:], is_transpose=True)
                nc.vector.tensor_copy(woT[:, ic, ec * 128:(ec + 1) * 128],
                                      ps[:])

        # Q shard: Q[j, e_sl] = sum_i wv[i, j] woT[i, e_sl]
        # wv loaded in two column halves like wq
        for h in range(2):
            wv_bf = p_w.tile([128, DC, HD], BF16, tag="wbig", name="wv_bf")
            for n in range(DC):
                wv_f = p_io.tile([128, HD], F32, tag="wload", bufs=2)
                nc.sync.dma_start(
                    wv_f[:], wv[n * 128:(n + 1) * 128, h * HD:(h + 1) * HD])
                if n % 2 == 0:
                    nc.scalar.activation(wv_bf[:, n, :], wv_f[:], CPY)
                else:
                    nc.vector.tensor_copy(wv_bf[:, n, :], wv_f[:])
            for jh in range(DC // 2):
                jc = h * (DC // 2) + jh
                ps = pp_m.tile([128, ES], F32)
                for ic in range(DC):
                    nc.tensor.matmul(ps[:],
                                     wv_bf[:, ic, jh * 128:(jh + 1) * 128],
                                     woT[:, ic, :], start=(ic == 0),
                                     stop=(ic == DC - 1))
                qstg = p_sm.tile([128, ES], BF16, tag="stg", bufs=2,
                                 name="qstg")
                if jc % 2 == 0:
                    nc.scalar.activation(qstg[:], ps[:], CPY)
                else:
                    nc.vector.tensor_copy(qstg[:], ps[:])
                nc.sync.dma_start(bnc2[jc * 128:(jc + 1) * 128, :], qstg[:])
        # c shard = 3*(bv @ wo_sl.T + bo_sl)
        ps_c = pp_r.tile([1, ES], F32, tag="ps256", name="ps_c")
        for ic in range(DC):
            nc.tensor.matmul(ps_c[:], bv_colbf[:, ic:ic + 1], woT[:, ic, :],
                             start=(ic == 0), stop=(ic == DC - 1))
        c_t = p_sm.tile([1, ES], F32)
        nc.vector.tensor_tensor(c_t[:], ps_c[:], bo_sl_row[:], op=ADD)
        cstg = p_sm.tile([1, ES], BF16)
        nc.vector.tensor_scalar(cstg[:], c_t[:], 3.0, None, op0=MULT)
        nc.sync.dma_start(bnc2[DC * 128:, :], cstg[:])
        nc.gpsimd.collective_compute(
            "AllGather", mybir.AluOpType.bypass,
            replica_groups=[list(range(NCORES))],
            ins=[bnc2[:]], outs=[ag2[:]])

        # assemble full M (bf16) and r from AG1
        for s in range(NCORES):
            nc.sync.dma_start(
                m_bf[:, :, s * ES:(s + 1) * ES],
                ag1[s, 0:DC * 128, :].rearrange("(c p) e -> p c e", p=128))
        for s in range(NCORES):
            r_sl = p_sm.tile([1, ES], BF16, tag="rowsl", bufs=2, name="r_sl")
            nc.sync.dma_start(r_sl[:], ag1[s, DC * 128:, :])
            nc.gpsimd.partition_broadcast(r_rep[:, s * ES:(s + 1) * ES],
                                          r_sl[:])

        # assemble full Q (bf16) and c from AG2
        for s in range(NCORES):
            nc.sync.dma_start(
                q_bf[:, :, s * ES:(s + 1) * ES],
                ag2[s, 0:DC * 128, :].rearrange("(c p) e -> p c e", p=128))
        for s in range(NCORES):
            c_sl = p_sm.tile([1, ES], BF16, tag="rowsl", bufs=2, name="c_sl")
            nc.sync.dma_start(c_sl[:], ag2[s, DC * 128:, :])
            nc.gpsimd.partition_broadcast(c_rep[:, s * ES:(s + 1) * ES],
                                          c_sl[:])

    # ------------- Phase 2: per b-tile scores/softmax/zv/y -----------------
    with (
        tc.tile_pool(name="p2_io", bufs=2) as p_io,
        tc.tile_pool(name="p2_zq", bufs=1) as p_zq,
        tc.tile_pool(name="p2_sc", bufs=1) as p_sc,
        tc.tile_pool(name="p2_pst", bufs=2, space="PSUM") as pp_t,
        tc.tile_pool(name="p2_psg", bufs=3, space="PSUM") as pp_g,
        tc.tile_pool(name="p2_psy", bufs=2, space="PSUM") as pp_y,
    ):
        def sec_a(ib):
            """z loads + casts (ACT) + zq transposes for tile ib"""
            r0 = ib * 128
            st = {}
            st["zq_bf"] = p_zq.tile([128, T, D], BF16, tag="zqbf", bufs=1,
                                    name="zq_bf")
            for t in range(T):
                zq_f = p_io.tile([128, D], F32, tag="zf", bufs=2, name="zq_f")
                nc.sync.dma_start(zq_f[:], z[r0:r0 + 128, t * D:(t + 1) * D])
                nc.scalar.activation(st["zq_bf"][:, t, :], zq_f[:], CPY)
            st["zk_bf"] = p_zq.tile([128, T, D], BF16, tag="zkbf", bufs=1,
                                    name="zk_bf")
            for s in range(T):
                zk_f = p_io.tile([128, D], F32, tag="zf", bufs=2, name="zk_f")
                nc.sync.dma_start(
                    zk_f[:], z[r0:r0 + 128, (T + s) * D:(T + s + 1) * D])
                # fold the 1/sqrt(D) score scale into the cast
                nc.scalar.activation(st["zk_bf"][:, s, :], zk_f[:], CPY,
                                     scale=SQD)
            st["zqT"] = p_zq.tile([128, T, DC, 128], BF16, tag="zqT", bufs=1,
                                  name="zqT")
            for t in range(T):
                for dg in range(DC // 8):
                    ps = pp_t.tile([128, 8, 128], BF16)
                    for j in range(8):
                        d = dg * 8 + j
                        nc.tensor.matmul(
                            ps[:, j, :],
                            st["zq_bf"][:, t, d * 128:(d + 1) * 128],
                            ident[:], is_transpose=True)
                    nc.vector.tensor_copy(
                        st["zqT"][:, t, dg * 8:(dg + 1) * 8, :], ps[:])
            return st

        def sec_cb(ib, st):
            """G matmuls with fused psum-direct dots, then softmax + zv"""
            sraw = p_sc.tile([128, T, T], F32, tag="sraw", bufs=1, name="sraw")
            spart = p_sc.tile([128, T, T, EC], F32, tag="spart", bufs=1,
                              name="spart")
            rho = p_sc.tile([128, T], F32, tag="rho", bufs=1, name="rho")
            scrb = p_io.tile([128, HD], BF16, tag="scrb", bufs=1,
                             name="scrb")
            rho2 = p_sc.tile([128, T, 2], F32, tag="rho2", bufs=1,
                             name="rho2")
            for s_ in range(T):
                for k in range(2):
                    nc.vector.tensor_tensor(
                        scrb[:], r_rep[:, k * HD:(k + 1) * HD],
                        st["zk_bf"][:, s_, k * HD:(k + 1) * HD], op=MULT)
                    nc.vector.tensor_reduce(rho2[:, s_, k:k + 1], scrb[:],
                                            axis=mybir.AxisListType.X, op=ADD)
            nc.vector.tensor_reduce(rho[:], rho2[:],
                                    axis=mybir.AxisListType.X, op=ADD)
            for t in range(T):
                for e in range(EC):
                    ps = pp_g.tile([128, 512], F32)
                    for d in range(DC):
                        nc.tensor.matmul(
                            ps[:], st["zqT"][:, t, d, :],
                            m_bf[:, d, e * 512:(e + 1) * 512],
                            start=(d == 0), stop=(d == DC - 1))
                    for s_ in range(T):
                        scr = p_io.tile([128, 512], BF16, tag="scr", bufs=2,
                                        name="scr")
                        nc.vector.tensor_tensor(
                            scr[:], ps[:],
                            st["zk_bf"][:, s_, e * 512:(e + 1) * 512],
                            op=MULT)
                        scr2 = p_io.tile([128, 512], BF16, tag="scr", bufs=2,
                                         name="scr2")
                        nc.scalar.activation(
                            scr2[:], scr[:], CPY,
                            accum_out=spart[:, t, s_, e:e + 1])
            sdot = p_sc.tile([128, T, T], F32, tag="sdot", bufs=1,
                             name="sdot")
            nc.vector.tensor_reduce(sdot[:], spart[:],
                                    axis=mybir.AxisListType.X, op=ADD)
            for t in range(T):
                nc.vector.tensor_tensor(sraw[:, t, :], sdot[:, t, :],
                                        rho[:], op=ADD)
            # a-dots, softmax, zv
            tvec = p_sc.tile([128, T], F32, tag="tvec", bufs=1)
            traw = p_sc.tile([128, T], F32, tag="traw", bufs=1)
            for t in range(T):
                for k in range(2):
                    nc.vector.tensor_tensor(
                        scrb[:], st["zq_bf"][:, t, k * HD:(k + 1) * HD],
                        a_rep[:, k * HD:(k + 1) * HD], op=MULT)
                    nc.vector.tensor_reduce(rho2[:, t, k:k + 1], scrb[:],
                                            axis=mybir.AxisListType.X, op=ADD)
            nc.vector.tensor_reduce(traw[:], rho2[:],
                                    axis=mybir.AxisListType.X, op=ADD)
            for t in range(T):
                nc.vector.tensor_scalar(tvec[:, t:t + 1], traw[:, t:t + 1],
                                        SQD / MSC, kap_col[:], op0=MULT,
                                        op1=ADD)
            p_un = p_sc.tile([128, T, T], F32, tag="p_un", bufs=1)
            nc.scalar.activation(p_un[:, 0, :], sraw[:, 0, :], EXP,
                                 bias=tvec[:, 0:1])
            nc.scalar.activation(p_un[:, 1, 1:], sraw[:, 1, 1:], EXP,
                                 bias=tvec[:, 1:2])
            nc.scalar.activation(p_un[:, 2, 2:], sraw[:, 2, 2:], EXP,
                                 bias=tvec[:, 2:3])
            nc.vector.memset(p_un[:, 1, 0:1], 1.0)
            nc.vector.memset(p_un[:, 2, 0:2], 1.0)
            rsum = p_sc.tile([128, T], F32, tag="rsum", bufs=1)
            nc.vector.tensor_reduce(rsum[:], p_un[:],
                                    axis=mybir.AxisListType.X, op=ADD)
            rinv = p_sc.tile([128, T], F32, tag="rinv", bufs=1)
            nc.vector.reciprocal(rinv[:], rsum[:])
            pn = p_sc.tile([128, T, T], F32, tag="pn", bufs=1)
            for t in range(T):
                nc.vector.tensor_scalar(pn[:, t, :], p_un[:, t, :],
                                        rinv[:, t:t + 1], None, op0=MULT)
            ws = p_sc.tile([128, T], F32, tag="ws", bufs=1)
            nc.vector.tensor_reduce(ws[:], pn.rearrange("p t s -> p s t"),
                                    axis=mybir.AxisListType.X, op=ADD)
            zv_bf = p_sc.tile([128, D], BF16, tag="zv", bufs=1)
            nc.vector.tensor_scalar(zv_bf[:], st["zq_bf"][:, 0, :], ws[:, 0:1],
                                    None, op0=MULT)
            nc.vector.scalar_tensor_tensor(zv_bf[:], st["zq_bf"][:, 1, :],
                                           ws[:, 1:2], zv_bf[:], op0=MULT,
                                           op1=ADD)
            nc.vector.scalar_tensor_tensor(zv_bf[:], st["zq_bf"][:, 2, :],
                                           ws[:, 2:3], zv_bf[:], op0=MULT,
                                           op1=ADD)
            st["zv"] = zv_bf

        def sec_d(ib, st):
            """transpose zv, then y = zvT.T @ Q + c straight from psum"""
            r0 = ib * 128
            zvT = p_sc.tile([128, DC, 128], BF16, tag="zvT", bufs=1)
            for dg in range(DC // 8):
                ps = pp_t.tile([128, 8, 128], BF16)
                for j in range(8):
                    d = dg * 8 + j
                    nc.tensor.matmul(ps[:, j, :],
                                     st["zv"][:, d * 128:(d + 1) * 128],
                                     ident[:], is_transpose=True)
                nc.vector.tensor_copy(zvT[:, dg * 8:(dg + 1) * 8, :], ps[:])
            for e in range(EC):
                ps = pp_y.tile([128, 512], F32)
                for jc in range(DC):
                    nc.tensor.matmul(
                        ps[:], zvT[:, jc, :],
                        q_bf[:, jc, e * 512:(e + 1) * 512],
                        start=(jc == 0), stop=(jc == DC - 1))
                y_sb = p_io.tile([128, 512], F32, tag="ysb", bufs=1,
                                 name="y_sb")
                nc.vector.tensor_tensor(y_sb[:], ps[:],
                                        c_rep[:, e * 512:(e + 1) * 512],
                                        op=ADD)
                nc.sync.dma_start(
                    out[r0:r0 + 128, e * 512:(e + 1) * 512], y_sb[:])

        state = [None] * NB
        state[0] = sec_a(0)
        for ib in range(NB):
            sec_cb(ib, state[ib])
            if ib + 1 < NB:
                state[ib + 1] = sec_a(ib + 1)
            sec_d(ib, state[ib])

    dram.release()
    qpool.release()
    persist.release()
    const.release()


def build_nc(b_loc):
    nc = bacc.Bacc("TRN2", target_bir_lowering=False, debug=False,
                   num_devices=NCORES)
    aps = {}
    aps["z"] = nc.dram_tensor("z", [b_loc, 2 * T * D], F32,
                              kind="ExternalInput").ap()
    for w in ("wq", "wv"):
        aps[w] = nc.dram_tensor(w, [D, D], F32, kind="ExternalInput").ap()
    aps["wk_sl"] = nc.dram_tensor("wk_sl", [D, ES], F32,
                                  kind="ExternalInput").ap()
    aps["wo_sl"] = nc.dram_tensor("wo_sl", [ES, D], F32,
                                  kind="ExternalInput").ap()
    for b_ in ("bq", "bk", "bv"):
        aps[b_] = nc.dram_tensor(b_, [D], F32, kind="ExternalInput").ap()
    aps["bo_sl"] = nc.dram_tensor("bo_sl", [ES], F32,
                                  kind="ExternalInput").ap()
    aps["out"] = nc.dram_tensor("out", [b_loc, D], F32,
                                kind="ExternalOutput").ap()
    with tile.TileContext(nc) as tc:
        emit(tc, aps, b_loc)
    nc.compile()
    return nc


_CACHE = {}


def _get_nc(b_loc):
    if b_loc not in _CACHE:
        _CACHE[b_loc] = build_nc(b_loc)
    return _CACHE[b_loc]


def make_in_maps(arrs):
    b_loc = B // NCORES
    in_maps = []
    for c in range(NCORES):
        m = {k: arrs[k] for k in ("wq", "wv", "bq", "bk", "bv")}
        m["wk_sl"] = np.ascontiguousarray(arrs["wk"][:, c * ES:(c + 1) * ES])
        m["wo_sl"] = np.ascontiguousarray(arrs["wo"][c * ES:(c + 1) * ES, :])
        m["bo_sl"] = np.ascontiguousarray(arrs["bo"][c * ES:(c + 1) * ES])
        m["z"] = arrs["z"][c * b_loc:(c + 1) * b_loc]
        in_maps.append(m)
    return in_maps


def kernel(**inputs):
    arrs = {k: np.ascontiguousarray(np.asarray(v, dtype=np.float32))
            for k, v in inputs.items()}
    b_loc = B // NCORES
    nc = _get_nc(b_loc)
    in_maps = make_in_maps(arrs)
    res = run_bass_kernel_spmd(nc, in_maps, core_ids=list(range(NCORES)))
    return np.concatenate([r["out"] for r in res.results], axis=0)


# revision 16
# speedup vs baseline: 1.0114x; 1.0114x over previous
"""Trainium2 Bass kernel for nn_Attention (sparse_attention, T=3).

Math (per batch row b, derived from the reference):
    zq = z[:, :3*2048].reshape(B, 3, D)   (q and v source)
    zk = z[:, 3*2048:].reshape(B, 3, D)
    scores[t,s] = (zq[t] @ M @ zk[s] + zq[t].a + r.zk[s] + kap) / sqrt(D)
      with M = wq.T @ wk, a = wq.T @ bk, r = bq @ wk, kap = bq.bk
    strictly-lower entries of scores are replaced by 0 before softmax
    p = softmax(scores); w[s] = sum_t p[t,s]
    y = (sum_s w[s]*zq[s]) @ Q + c,  Q = wv.T @ wo.T,  c = 3*bv@wo.T + 3*bo

Distribution: data-parallel over batch (B/8 rows per core), plus the two
DxD weight products M and Q are column-sharded: core c computes the
256-wide e-slice it was fed (host passes wk[:, c*256:+256] and
wo[c*256:+256, :]), then an 8-core AllGather replicates the full M and
Q in bf16. Score dots are fused multiply-reduce (DVE) reading the G
psums directly; the value path is a single matmul against Q.
"""

import sys

sys.path.insert(0, "/opt/trn_rl_repo")

import numpy as np
from concourse import bacc, bass, masks, mybir, tile
from concourse.bass_utils import run_bass_kernel_spmd

F32 = mybir.dt.float32
BF16 = mybir.dt.bfloat16
FP8 = mybir.dt.float8e4
ADD = mybir.AluOpType.add
MULT = mybir.AluOpType.mult
CPY = mybir.ActivationFunctionType.Copy
EXP = mybir.ActivationFunctionType.Exp

B = 8192
D = 2048
T = 3
NCORES = 8
DC = D // 128       # 16 chunks of the feature dim
EC = D // 512       # 4 psum-wide slices
ES = D // NCORES    # 256: per-core e-shard width
HD = D // 2
SQD = 1.0 / float(np.sqrt(np.float32(D)))
MSC = 32.0          # fp8 scaling for the a vector


def emit(tc, aps, b_loc):
    nc = tc.nc
    z, wq, wk_sl, wv, wo_sl = (aps["z"], aps["wq"], aps["wk_sl"],
                               aps["wv"], aps["wo_sl"])
    bq, bk, bv, bo_sl, out = (aps["bq"], aps["bk"], aps["bv"], aps["bo_sl"],
                              aps["out"])
    NB = b_loc // 128

    const = tc.alloc_tile_pool(name="const", bufs=1)
    persist = tc.alloc_tile_pool(name="persist", bufs=1)

    ident = const.tile([128, 128], BF16)
    masks.make_identity(nc, ident[:])

    # biases in column layout: col[p, c] = vec[c*128 + p]
    bq_col = const.tile([128, DC], F32)
    bk_col = const.tile([128, DC], F32)
    bv_col = const.tile([128, DC], F32)
    bo_sl_row = const.tile([1, ES], F32)
    nc.sync.dma_start(bq_col[:], bq.rearrange("(c p) -> p c", p=128))
    nc.sync.dma_start(bk_col[:], bk.rearrange("(c p) -> p c", p=128))
    nc.sync.dma_start(bv_col[:], bv.rearrange("(c p) -> p c", p=128))
    nc.sync.dma_start(bo_sl_row[:], bo_sl[None, :])
    nc.vector.tensor_scalar(bo_sl_row[:], bo_sl_row[:], 3.0, None, op0=MULT)
    bq_colbf = const.tile([128, DC], BF16)
    bk_colbf = const.tile([128, DC], BF16)
    bv_colbf = const.tile([128, DC], BF16)
    nc.vector.tensor_copy(bq_colbf[:], bq_col[:])
    nc.vector.tensor_copy(bk_colbf[:], bk_col[:])
    nc.vector.tensor_copy(bv_colbf[:], bv_col[:])

    # broadcast score-bias tensors (filled in phase 1)
    a_rep = persist.tile([128, D], FP8)     # 32*a = 32*wq.T@bk, replicated
    r_rep = persist.tile([128, D], FP8)     # 32*r = 32*bq@wk, replicated
    c_rep = persist.tile([128, D], BF16)    # c = 3bv@wo.T + 3bo, replicated
    kap_col = persist.tile([128, 1], F32)   # bq.bk / sqrt(D)

    m_bf = persist.tile([128, DC, D], BF16)  # M[d, e], partition = d%128
    qpool = tc.alloc_tile_pool(name="qpool", bufs=1, side="right")
    q_bf = qpool.tile([128, DC, D], BF16)   # Q[j, e], partition = j%128

    # ------------- Phase 1: sharded M/Q products + AllGather ---------------
    dram = tc.alloc_tile_pool(name="dram", bufs=1, space="DRAM")
    bnc1 = dram.tile([DC * 128 + 1, ES], BF16)
    ag1 = dram.tile([NCORES, DC * 128 + 1, ES], BF16, addr_space="Shared")
    bnc2 = dram.tile([DC * 128 + 1, ES], BF16)
    ag2 = dram.tile([NCORES, DC * 128 + 1, ES], BF16, addr_space="Shared")

    with (
        tc.tile_pool(name="p1_w", bufs=1) as p_w,
        tc.tile_pool(name="p1_io", bufs=2) as p_io,
        tc.tile_pool(name="p1_sm", bufs=1) as p_sm,
        tc.tile_pool(name="p1_ps", bufs=3, space="PSUM") as pp_m,
        tc.tile_pool(name="p1_psr", bufs=1, space="PSUM") as pp_r,
    ):
        # wk shard in bf16 (rhs)
        wk_bf = p_sm.tile([128, DC, ES], BF16, tag="wsl", name="wk_bf")
        for n in range(DC):
            wk_f = p_io.tile([128, ES], F32, tag="wkload", bufs=1)
            nc.sync.dma_start(wk_f[:], wk_sl[n * 128:(n + 1) * 128, :])
            nc.scalar.activation(wk_bf[:, n, :], wk_f[:], CPY)

        # M shard: M[d, e_sl] = sum_n wq[n, d] wk[n, e_sl].
        # wq is loaded in two column halves (lhsT free dim) to halve SBUF.
        a_row = p_sm.tile([1, D], FP8, name="a_row")
        for h in range(2):
            wq_bf = p_w.tile([128, DC, HD], BF16, tag="wbig", name="wq_bf")
            for n in range(DC):
                for q4 in range(2):
                    wq_f = p_io.tile([128, 512], F32, tag="wload", bufs=6)
                    nc.sync.dma_start(
                        wq_f[:], wq[n * 128:(n + 1) * 128,
                                    h * HD + q4 * 512:h * HD + (q4 + 1) * 512])
                    sl = slice(q4 * 512, (q4 + 1) * 512)
                    if (2 * n + q4) % 2 == 0:
                        nc.scalar.activation(wq_bf[:, n, sl], wq_f[:], CPY)
                    else:
                        nc.vector.tensor_copy(wq_bf[:, n, sl], wq_f[:])
            for dh in range(DC // 2):
                dc = h * (DC // 2) + dh
                ps = pp_m.tile([128, ES], F32)
                for n in range(DC):
                    nc.tensor.matmul(ps[:],
                                     wq_bf[:, n, dh * 128:(dh + 1) * 128],
                                     wk_bf[:, n, :], start=(n == 0),
                                     stop=(n == DC - 1))
                mstg = p_sm.tile([128, ES], BF16, tag="stg", bufs=2,
                                 name="mstg")
                nc.vector.tensor_copy(mstg[:], ps[:])
                nc.sync.dma_start(bnc1[dc * 128:(dc + 1) * 128, :], mstg[:])
            # a = wq.T @ bk for this half (x32, fp8)
            for e in range(EC // 2):
                ps_a = pp_r.tile([1, 512], F32, tag="psa")
                for n in range(DC):
                    nc.tensor.matmul(ps_a[:], bk_colbf[:, n:n + 1],
                                     wq_bf[:, n, e * 512:(e + 1) * 512],
                                     start=(n == 0), stop=(n == DC - 1))
                nc.vector.tensor_scalar(
                    a_row[:, h * HD + e * 512:h * HD + (e + 1) * 512],
                    ps_a[:], MSC, None, op0=MULT)
        # r shard = bq @ wk
        ps_r = pp_r.tile([1, ES], F32, tag="ps256", name="ps_r")
        for n in range(DC):
            nc.tensor.matmul(ps_r[:], bq_colbf[:, n:n + 1], wk_bf[:, n, :],
                             start=(n == 0), stop=(n == DC - 1))
        rstg = p_sm.tile([1, ES], BF16)
        nc.vector.tensor_copy(rstg[:], ps_r[:])
        nc.sync.dma_start(bnc1[DC * 128:, :], rstg[:])
        nc.gpsimd.collective_compute(
            "AllGather", mybir.AluOpType.bypass,
            replica_groups=[list(range(NCORES))],
            ins=[bnc1[:]], outs=[ag1[:]])

        nc.gpsimd.partition_broadcast(a_rep[:], a_row[:])
        ps_k = pp_r.tile([1, ES], F32, tag="ps256", name="ps_k")
        for n in range(DC):
            nc.tensor.matmul(ps_k[:, 0:1], bq_colbf[:, n:n + 1],
                             bk_colbf[:, n:n + 1],
                             start=(n == 0), stop=(n == DC - 1))
        kap_row = p_sm.tile([1, 1], F32)
        nc.vector.tensor_copy(kap_row[:], ps_k[:, 0:1])
        nc.gpsimd.partition_broadcast(kap_col[:], kap_row[:])
        nc.vector.tensor_scalar(kap_col[:], kap_col[:], SQD, None, op0=MULT)

        # wo shard rows, transposed on PE: woT[i, e_sl]
        woT = p_sm.tile([128, DC, ES], BF16, tag="wsl", name="woT")
        for ec in range(ES // 128):
            wo_f = p_io.tile([128, D], F32, tag="wold", bufs=1)
            nc.sync.dma_start(wo_f[:], wo_sl[ec * 128:(ec + 1) * 128, :])
            wo_b = p_io.tile([128, D], BF16, tag="wob", bufs=1)
            nc.scalar.activation(wo_b[:], wo_f[:], CPY)
            for ic in range(DC):
                ps = pp_m.tile([128, 128], BF16, tag="pst", bufs=2)
                nc.tensor.matmul(ps[:], wo_b[:, ic * 128:(ic + 1) * 128],
                                 ident[:], is_transpose=True)
                nc.vector.tensor_copy(woT[:, ic, ec * 128:(ec + 1) * 128],
                                      ps[:])

        # Q shard: Q[j, e_sl] = sum_i wv[i, j] woT[i, e_sl]
        # wv loaded in two column halves like wq
        for h in range(2):
            wv_bf = p_w.tile([128, DC, HD], BF16, tag="wbig", name="wv_bf")
            for n in range(DC):
                for q4 in range(2):
                    wv_f = p_io.tile([128, 512], F32, tag="wload", bufs=6)
                    nc.sync.dma_start(
                        wv_f[:], wv[n * 128:(n + 1) * 128,
                                    h * HD + q4 * 512:h * HD + (q4 + 1) * 512])
                    sl = slice(q4 * 512, (q4 + 1) * 512)
                    if (2 * n + q4) % 2 == 0:
                        nc.scalar.activation(wv_bf[:, n, sl], wv_f[:], CPY)
                    else:
                        nc.vector.tensor_copy(wv_bf[:, n, sl], wv_f[:])
            for jh in range(DC // 2):
                jc = h * (DC // 2) + jh
                ps = pp_m.tile([128, ES], F32)
                for ic in range(DC):
                    nc.tensor.matmul(ps[:],
                                     wv_bf[:, ic, jh * 128:(jh + 1) * 128],
                                     woT[:, ic, :], start=(ic == 0),
                                     stop=(ic == DC - 1))
                qstg = p_sm.tile([128, ES], BF16, tag="stg", bufs=2,
                                 name="qstg")
                if jc % 2 == 0:
                    nc.scalar.activation(qstg[:], ps[:], CPY)
                else:
                    nc.vector.tensor_copy(qstg[:], ps[:])
                nc.sync.dma_start(bnc2[jc * 128:(jc + 1) * 128, :], qstg[:])
        # c shard = 3*(bv @ wo_sl.T + bo_sl)
        ps_c = pp_r.tile([1, ES], F32, tag="ps256", name="ps_c")
        for ic in range(DC):
            nc.tensor.matmul(ps_c[:], bv_colbf[:, ic:ic + 1], woT[:, ic, :],
                             start=(ic == 0), stop=(ic == DC - 1))
        cstg = p_sm.tile([1, ES], BF16)
        nc.vector.scalar_tensor_tensor(cstg[:], ps_c[:], 3.0, bo_sl_row[:],
                                       op0=MULT, op1=ADD)
        nc.sync.dma_start(bnc2[DC * 128:, :], cstg[:])
        nc.gpsimd.collective_compute(
            "AllGather", mybir.AluOpType.bypass,
            replica_groups=[list(range(NCORES))],
            ins=[bnc2[:]], outs=[ag2[:]])

        # assemble full M (bf16) and r from AG1
        for s in range(NCORES):
            for hh in range(2):
                nc.sync.dma_start(
                    m_bf[:, hh * 8:(hh + 1) * 8, s * ES:(s + 1) * ES],
                    ag1[s, hh * 1024:(hh + 1) * 1024, :]
                    .rearrange("(c p) e -> p c e", p=128))
        for s in range(NCORES):
            r_sl = p_sm.tile([1, ES], BF16, tag="rowsl", bufs=1, name="r_sl")
            nc.sync.dma_start(r_sl[:], ag1[s, DC * 128:, :])
            r_sl8 = p_sm.tile([1, ES], FP8, tag="rowsl8", bufs=1, name="r_sl8")
            nc.vector.tensor_scalar(r_sl8[:], r_sl[:], MSC, None, op0=MULT)
            nc.gpsimd.partition_broadcast(r_rep[:, s * ES:(s + 1) * ES],
                                          r_sl8[:])

        # assemble full Q (bf16) and c from AG2
        for s in range(NCORES):
            for hh in range(2):
                nc.sync.dma_start(
                    q_bf[:, hh * 8:(hh + 1) * 8, s * ES:(s + 1) * ES],
                    ag2[s, hh * 1024:(hh + 1) * 1024, :]
                    .rearrange("(c p) e -> p c e", p=128))
        for s in range(NCORES):
            c_sl = p_sm.tile([1, ES], BF16, tag="rowsl", bufs=1, name="c_sl")
            nc.sync.dma_start(c_sl[:], ag2[s, DC * 128:, :])
            nc.gpsimd.partition_broadcast(c_rep[:, s * ES:(s + 1) * ES],
                                          c_sl[:])

    # ------------- Phase 2: per b-tile scores/softmax/zv/y -----------------
    with (
        tc.tile_pool(name="p2_io", bufs=2) as p_io,
        tc.tile_pool(name="p2_zq", bufs=1) as p_zq,
        tc.tile_pool(name="p2_sc", bufs=1) as p_sc,
        tc.tile_pool(name="p2_pst", bufs=2, space="PSUM") as pp_t,
        tc.tile_pool(name="p2_psg", bufs=4, space="PSUM") as pp_g,
        tc.tile_pool(name="p2_psy", bufs=2, space="PSUM") as pp_y,
    ):
        def sec_a(ib):
            """z loads + casts (ACT) + zq transposes for tile ib"""
            r0 = ib * 128
            st = {}
            st["zq_bf"] = p_zq.tile([128, T, D], BF16, tag="zqbf", bufs=1,
                                    name="zq_bf")
            for t in range(T):
                zq_f = p_io.tile([128, D], F32, tag="zf", bufs=2, name="zq_f")
                nc.sync.dma_start(zq_f[:], z[r0:r0 + 128, t * D:(t + 1) * D])
                nc.scalar.activation(st["zq_bf"][:, t, :], zq_f[:], CPY)
            st["zk_bf"] = p_zq.tile([128, T, D], BF16, tag="zkbf", bufs=1,
                                    name="zk_bf")
            for s in range(T):
                zk_f = p_io.tile([128, D], F32, tag="zf", bufs=2, name="zk_f")
                nc.sync.dma_start(
                    zk_f[:], z[r0:r0 + 128, (T + s) * D:(T + s + 1) * D])
                # fold the 1/sqrt(D) score scale into the cast
                nc.scalar.activation(st["zk_bf"][:, s, :], zk_f[:], CPY,
                                     scale=SQD)
            st["zqT"] = p_zq.tile([128, T, DC, 128], BF16, tag="zqT", bufs=1,
                                  name="zqT")
            for t in range(T):
                for dg in range(DC // 8):
                    ps = pp_t.tile([128, 8, 128], BF16)
                    for j in range(8):
                        d = dg * 8 + j
                        nc.tensor.matmul(
                            ps[:, j, :],
                            st["zq_bf"][:, t, d * 128:(d + 1) * 128],
                            ident[:], is_transpose=True)
                    nc.vector.tensor_copy(
                        st["zqT"][:, t, dg * 8:(dg + 1) * 8, :], ps[:])
            return st

        def sec_cb(ib, st):
            """G matmuls with fused psum-direct dots, then softmax + zv"""
            sraw = p_sc.tile([128, T, T], F32, tag="sraw", bufs=1, name="sraw")
            spart = p_sc.tile([128, T, T, EC], F32, tag="spart", bufs=1,
                              name="spart")
            rho = p_sc.tile([128, T], F32, tag="rho", bufs=1, name="rho")
            scrb = p_io.tile([128, HD], BF16, tag="scrb", bufs=1,
                             name="scrb")
            rho2 = p_sc.tile([128, T, 2], F32, tag="rho2", bufs=1,
                             name="rho2")
            for s_ in range(T):
                for k in range(2):
                    nc.vector.tensor_tensor(
                        scrb[:], r_rep[:, k * HD:(k + 1) * HD],
                        st["zk_bf"][:, s_, k * HD:(k + 1) * HD], op=MULT)
                    nc.vector.tensor_reduce(rho2[:, s_, k:k + 1], scrb[:],
                                            axis=mybir.AxisListType.X, op=ADD)
            nc.vector.tensor_reduce(rho[:], rho2[:],
                                    axis=mybir.AxisListType.X, op=ADD)
            nc.vector.tensor_scalar(rho[:], rho[:], 1.0 / MSC, None, op0=MULT)
            for t in range(T):
                for e in range(EC):
                    ps = pp_g.tile([128, 512], F32)
                    for d in range(DC):
                        nc.tensor.matmul(
                            ps[:], st["zqT"][:, t, d, :],
                            m_bf[:, d, e * 512:(e + 1) * 512],
                            start=(d == 0), stop=(d == DC - 1))
                    for s_ in range(T):
                        scr = p_io.tile([128, 512], BF16, tag="scr", bufs=2,
                                        name="scr")
                        nc.vector.tensor_tensor(
                            scr[:], ps[:],
                            st["zk_bf"][:, s_, e * 512:(e + 1) * 512],
                            op=MULT)
                        nc.vector.tensor_reduce(
                            spart[:, t, s_, e:e + 1], scr[:],
                            axis=mybir.AxisListType.X, op=ADD)
            sdot = p_sc.tile([128, T, T], F32, tag="sdot", bufs=1,
                             name="sdot")
            nc.vector.tensor_reduce(sdot[:], spart[:],
                                    axis=mybir.AxisListType.X, op=ADD)
            for t in range(T):
                nc.vector.tensor_tensor(sraw[:, t, :], sdot[:, t, :],
                                        rho[:], op=ADD)
            # a-dots, softmax, zv
            tvec = p_sc.tile([128, T], F32, tag="tvec", bufs=1)
            traw = p_sc.tile([128, T], F32, tag="traw", bufs=1)
            for t in range(T):
                for k in range(2):
                    nc.vector.tensor_tensor(
                        scrb[:], st["zq_bf"][:, t, k * HD:(k + 1) * HD],
                        a_rep[:, k * HD:(k + 1) * HD], op=MULT)
                    nc.vector.tensor_reduce(rho2[:, t, k:k + 1], scrb[:],
                                            axis=mybir.AxisListType.X, op=ADD)
            nc.vector.tensor_reduce(traw[:], rho2[:],
                                    axis=mybir.AxisListType.X, op=ADD)
            for t in range(T):
                nc.vector.tensor_scalar(tvec[:, t:t + 1], traw[:, t:t + 1],
                                        SQD / MSC, kap_col[:], op0=MULT,
                                        op1=ADD)
            p_un = p_sc.tile([128, T, T], F32, tag="p_un", bufs=1)
            nc.scalar.activation(p_un[:, 0, :], sraw[:, 0, :], EXP,
                                 bias=tvec[:, 0:1])
            nc.scalar.activation(p_un[:, 1, 1:], sraw[:, 1, 1:], EXP,
                                 bias=tvec[:, 1:2])
            nc.scalar.activation(p_un[:, 2, 2:], sraw[:, 2, 2:], EXP,
                                 bias=tvec[:, 2:3])
            nc.vector.memset(p_un[:, 1, 0:1], 1.0)
            nc.vector.memset(p_un[:, 2, 0:2], 1.0)
            rsum = p_sc.tile([128, T], F32, tag="rsum", bufs=1)
            nc.vector.tensor_reduce(rsum[:], p_un[:],
                                    axis=mybir.AxisListType.X, op=ADD)
            rinv = p_sc.tile([128, T], F32, tag="rinv", bufs=1)
            nc.vector.reciprocal(rinv[:], rsum[:])
            pn = p_sc.tile([128, T, T], F32, tag="pn", bufs=1)
            for t in range(T):
                nc.vector.tensor_scalar(pn[:, t, :], p_un[:, t, :],
                                        rinv[:, t:t + 1], None, op0=MULT)
            ws = p_sc.tile([128, T], F32, tag="ws", bufs=1)
            nc.vector.tensor_reduce(ws[:], pn.rearrange("p t s -> p s t"),
                                    axis=mybir.AxisListType.X, op=ADD)
            zv_bf = p_sc.tile([128, D], BF16, tag="zv", bufs=1)
            nc.vector.tensor_scalar(zv_bf[:], st["zq_bf"][:, 0, :], ws[:, 0:1],
                                    None, op0=MULT)
            nc.vector.scalar_tensor_tensor(zv_bf[:], st["zq_bf"][:, 1, :],
                                           ws[:, 1:2], zv_bf[:], op0=MULT,
                                           op1=ADD)
            nc.vector.scalar_tensor_tensor(zv_bf[:], st["zq_bf"][:, 2, :],
                                           ws[:, 2:3], zv_bf[:], op0=MULT,
                                           op1=ADD)
            st["zv"] = zv_bf

        def sec_d(ib, st):
            """transpose zv, then y = zvT.T @ Q + c straight from psum"""
            r0 = ib * 128
            zvT = p_sc.tile([128, DC, 128], BF16, tag="zvT", bufs=1)
            for dg in range(DC // 8):
                ps = pp_t.tile([128, 8, 128], BF16)
                for j in range(8):
                    d = dg * 8 + j
                    nc.tensor.matmul(ps[:, j, :],
                                     st["zv"][:, d * 128:(d + 1) * 128],
                                     ident[:], is_transpose=True)
                nc.vector.tensor_copy(zvT[:, dg * 8:(dg + 1) * 8, :], ps[:])
            for e in range(EC):
                ps = pp_y.tile([128, 512], F32)
                for jc in range(DC):
                    nc.tensor.matmul(
                        ps[:], zvT[:, jc, :],
                        q_bf[:, jc, e * 512:(e + 1) * 512],
                        start=(jc == 0), stop=(jc == DC - 1))
                y_sb = p_io.tile([128, 512], F32, tag="ysb", bufs=1,
                                 name="y_sb")
                nc.vector.tensor_tensor(y_sb[:], ps[:],
                                        c_rep[:, e * 512:(e + 1) * 512],
                                        op=ADD)
                nc.sync.dma_start(
                    out[r0:r0 + 128, e * 512:(e + 1) * 512], y_sb[:])

        state = [None] * NB
        state[0] = sec_a(0)
        for ib in range(NB):
            sec_cb(ib, state[ib])
            if ib + 1 < NB:
                state[ib + 1] = sec_a(ib + 1)
            sec_d(ib, state[ib])

    dram.release()
    qpool.release()
    persist.release()
    const.release()


def build_nc(b_loc):
    nc = bacc.Bacc("TRN2", target_bir_lowering=False, debug=False,
                   num_devices=NCORES)
    aps = {}
    aps["z"] = nc.dram_tensor("z", [b_loc, 2 * T * D], F32,
                              kind="ExternalInput").ap()
    for w in ("wq", "wv"):
        aps[w] = nc.dram_tensor(w, [D, D], F32, kind="ExternalInput").ap()
    aps["wk_sl"] = nc.dram_tensor("wk_sl", [D, ES], F32,
                                  kind="ExternalInput").ap()
    aps["wo_sl"] = nc.dram_tensor("wo_sl", [ES, D], F32,
                                  kind="ExternalInput").ap()
    for b_ in ("bq", "bk", "bv"):
        aps[b_] = nc.dram_tensor(b_, [D], F32, kind="ExternalInput").ap()
    aps["bo_sl"] = nc.dram_tensor("bo_sl", [ES], F32,
                                  kind="ExternalInput").ap()
    aps["out"] = nc.dram_tensor("out", [b_loc, D], F32,
                                kind="ExternalOutput").ap()
    with tile.TileContext(nc) as tc:
        emit(tc, aps, b_loc)
    nc.compile()
    return nc


_CACHE = {}


def _get_nc(b_loc):
    if b_loc not in _CACHE:
        _CACHE[b_loc] = build_nc(b_loc)
    return _CACHE[b_loc]


def make_in_maps(arrs):
    b_loc = B // NCORES
    in_maps = []
    for c in range(NCORES):
        m = {k: arrs[k] for k in ("wq", "wv", "bq", "bk", "bv")}
        m["wk_sl"] = np.ascontiguousarray(arrs["wk"][:, c * ES:(c + 1) * ES])
        m["wo_sl"] = np.ascontiguousarray(arrs["wo"][c * ES:(c + 1) * ES, :])
        m["bo_sl"] = np.ascontiguousarray(arrs["bo"][c * ES:(c + 1) * ES])
        m["z"] = arrs["z"][c * b_loc:(c + 1) * b_loc]
        in_maps.append(m)
    return in_maps


def kernel(**inputs):
    arrs = {k: np.ascontiguousarray(np.asarray(v, dtype=np.float32))
            for k, v in inputs.items()}
    b_loc = B // NCORES
    nc = _get_nc(b_loc)
    in_maps = make_in_maps(arrs)
    res = run_bass_kernel_spmd(nc, in_maps, core_ids=list(range(NCORES)))
    return np.concatenate([r["out"] for r in res.results], axis=0)
